# revision 15
# baseline (speedup 1.0000x reference)
"""GCN 2-layer kernel for Trainium2, 8 NeuronCores — fused single launch,
gpsimd ap_gather message aggregation (v3).

out = log_softmax(Ahat @ relu(Ahat @ (x@W1) + b1) @ W2 + b2),
Ahat = D^-1/2 (A+I) D^-1/2, dinv folded into per-node pre/post scales.

Layout: feature-major ("transposed") tables. Per core m (12500 own nodes,
padded to RTL=12544 positions):
  phase A: g1 = dinv*(x@W1) [128 nodes, 16] per tile -> transpose -> write
           g1t [16, RTL] (feature-major, natural node order, pads zeroed)
  AllGather g1t -> tab1t [128, RTL] (partition 16m+h = core m feature h)
  layer-1 aggregation: for each region r (2 cores = 25088 columns):
     load region table rt [128, 25088] (8 group-replicated copies)
     for each dd-bucket: one gpsimd ap_gather (per-group independent int16
     indices, ELL with per-bucket K) -> msgs [128, SP*K] -> DVE strided
     reduce -> accumulate into acc [128, 1568]
     (partition 16g+h = feature h of dst block g; dst blocks = degree-rank
      round-robin so the 8 groups are load-balanced)
  post: g2 = dinv*relu(dinv*s1 + b1), write g2t [16, RTL] (block-major
        permuted order), AllGather -> tab2t, repeat aggregation with ix2
  phase C: a2 = dinv*s2; per group g: copy a2 slice to partitions 0:16,
        W2 matmul -> [40, cols] -> PE transpose -> [128, 40] + b2 ->
        log_softmax -> out rows (block-major order; host unpermutes).

Graph prep (degree sort, ELL index tables, dinv) happens on host.
"""
import sys
sys.path.insert(0, "/opt/trn_rl_repo")
import numpy as np

import concourse.bass as bass
import concourse.bacc as bacc
import concourse.mybir as mybir
import concourse.tile as tile
import concourse.bass_utils as bass_utils
from concourse.masks import make_identity

F32 = mybir.dt.float32
I16 = mybir.dt.int16

M_CORES = 8
NGRP = 8            # gpsimd 16-partition groups == dst blocks
NREG = 4            # table regions (2 cores each; ap_gather int16 limit)
CAP = 2048          # max num_idxs per ap_gather (SP*K), 8KB f32 msgs
SPMAX = 512         # max bucket span (tmp tile width)


def _make_buckets(SP_all, Kmat):
    """Adaptive dd-buckets: spans (start, span, K[r]) with span*K <= CAP.

    Kmat: [NREG, BL] per-dd-position max count (over groups) per region.
    Returns list of (dd0, span, [K per region]).
    """
    BL = Kmat.shape[1]
    buckets = []
    dd = 0
    while dd < BL:
        span = 4
        while True:
            nxt = min(span * 2, BL - dd, SPMAX)
            if nxt == span:
                break
            Ks = [int(Kmat[r, dd:dd + nxt].max()) for r in range(NREG)]
            if max(1, max(Ks)) * nxt > CAP or nxt % 4 != 0:
                break
            span = nxt
        Ks = [max(1, int(Kmat[r, dd:dd + span].max())) for r in range(NREG)]
        buckets.append((dd, span, Ks))
        dd += span
    return buckets


def _build_v3(NT, NPC, D_IN, H, C, buckets, n_cores=M_CORES):
    RTL = NT * 128
    BL = RTL // NGRP
    REG = 2 * RTL
    KD = D_IN // 128
    # idx column layout (int16 cols per bucket-region)
    XC = sum(sp * k // 16 for dd0, sp, Ks in buckets for k in Ks)

    nc = bacc.Bacc("TRN2", target_bir_lowering=False, debug=False,
                   num_devices=n_cores)
    x_ap = nc.dram_tensor("x", [RTL, D_IN], F32, kind="ExternalInput").ap()
    w1_ap = nc.dram_tensor("w1", [128, KD * H], F32, kind="ExternalInput").ap()
    w2_ap = nc.dram_tensor("w2", [H, C], F32, kind="ExternalInput").ap()
    b1c_ap = nc.dram_tensor("b1c", [128, 1], F32, kind="ExternalInput").ap()
    b2_ap = nc.dram_tensor("b2", [128, C], F32, kind="ExternalInput").ap()
    dvn_ap = nc.dram_tensor("dvn", [128, NT], F32, kind="ExternalInput").ap()
    dvb_ap = nc.dram_tensor("dvb", [128, BL], F32, kind="ExternalInput").ap()
    ix1_ap = nc.dram_tensor("ix1", [128, XC], I16, kind="ExternalInput").ap()
    ix2_ap = nc.dram_tensor("ix2", [128, XC], I16, kind="ExternalInput").ap()
    out_ap = nc.dram_tensor("out", [RTL, C], F32, kind="ExternalOutput").ap()

    g1t_ap = nc.dram_tensor("g1t", [H, RTL], F32, kind="Internal").ap()
    g2t_ap = nc.dram_tensor("g2t", [H, RTL], F32, kind="Internal").ap()
    tab1_ap = nc.dram_tensor("tab1t", [128, RTL], F32, kind="Internal",
                             addr_space="Shared").ap()
    tab2_ap = nc.dram_tensor("tab2t", [128, RTL], F32, kind="Internal",
                             addr_space="Shared").ap()
    rg = [list(range(n_cores))]

    with tile.TileContext(nc) as tc:
        with tc.tile_pool(name="const", bufs=1) as cpool, \
             tc.tile_pool(name="xin", bufs=3) as xpool, \
             tc.tile_pool(name="xt", bufs=4) as xtpool, \
             tc.tile_pool(name="rtab", bufs=1) as rpool, \
             tc.tile_pool(name="msg", bufs=2) as mpool, \
             tc.tile_pool(name="tmpp", bufs=2) as tpool, \
             tc.tile_pool(name="accs", bufs=1) as apool, \
             tc.tile_pool(name="psA", bufs=2, space="PSUM") as psA, \
             tc.tile_pool(name="psT", bufs=2, space="PSUM") as psT:
            ident = cpool.tile([128, 128], F32)
            make_identity(nc, ident[:])
            w1_t = cpool.tile([128, KD * H], F32)
            nc.sync.dma_start(out=w1_t[:], in_=w1_ap[:])
            w2_t = cpool.tile([H, C], F32)
            nc.sync.dma_start(out=w2_t[:], in_=w2_ap[:])
            b1c_t = cpool.tile([128, 1], F32)
            nc.sync.dma_start(out=b1c_t[:], in_=b1c_ap[:])
            b2_t = cpool.tile([128, C], F32)
            nc.sync.dma_start(out=b2_t[:], in_=b2_ap[:])
            dvn_t = cpool.tile([128, NT], F32)
            nc.sync.dma_start(out=dvn_t[:], in_=dvn_ap[:])
            dvb_t = cpool.tile([128, BL], F32)
            nc.sync.dma_start(out=dvb_t[:], in_=dvb_ap[:])
            ix1_t = cpool.tile([128, XC], I16)
            nc.sync.dma_start(out=ix1_t[:], in_=ix1_ap[:])
            ix2_t = cpool.tile([128, XC], I16)
            nc.sync.dma_start(out=ix2_t[:], in_=ix2_ap[:])

            # ---- phase A: g1t = (dinv * (x @ W1))^T --------------------
            for t in range(NT):
                xt_ = xpool.tile([128, D_IN], F32, tag="x")
                nc.sync.dma_start(out=xt_[:], in_=x_ap[t * 128:(t + 1) * 128, :])
                acc = psA.tile([128, H], F32, tag="accA")
                for k in range(KD):
                    ptr = psT.tile([128, 128], F32, tag="ptr")
                    nc.tensor.transpose(
                        out=ptr[:], in_=xt_[:, k * 128:(k + 1) * 128],
                        identity=ident[:])
                    xT = xtpool.tile([128, 128], F32, tag="xT")
                    nc.any.tensor_copy(xT[:], ptr[:])
                    nc.tensor.matmul(
                        out=acc[:], lhsT=xT[:],
                        rhs=w1_t[:, k * H:(k + 1) * H],
                        start=(k == 0), stop=(k == KD - 1))
                gt = xtpool.tile([128, H], F32, tag="gout")
                nc.vector.tensor_scalar_mul(gt[:], acc[:], dvn_t[:, t:t + 1])
                ptg = psT.tile([128, 128], F32, tag="ptr")
                nc.tensor.transpose(out=ptg[:H, :], in_=gt[:, :], identity=ident[:])
                gT = xtpool.tile([H, 128], F32, tag="gT")
                nc.any.tensor_copy(gT[:], ptg[:H, :])
                nc.sync.dma_start(out=g1t_ap[:, t * 128:(t + 1) * 128], in_=gT[:])
            npad = RTL - NPC
            zp = xtpool.tile([H, max(npad, 1)], F32, tag="zp")
            nc.vector.memset(zp[:], 0.0)
            nc.sync.dma_start(out=g1t_ap[:, NPC:RTL], in_=zp[:, :npad])

            nc.gpsimd.collective_compute(
                "AllGather", mybir.AluOpType.bypass, replica_groups=rg,
                ins=[g1t_ap[:, :].opt()], outs=[tab1_ap[:, :].opt()])

            # ---- aggregation layers ------------------------------------
            def aggregate(tab_ap, ix_t, layer):
                acc1 = apool.tile([128, BL], F32, tag=f"acc{layer}")
                nc.vector.memset(acc1[:], 0.0)
                off = 0
                for r in range(NREG):
                    rt = rpool.tile([128, REG], F32, tag="rt")
                    for g in range(NGRP):
                        for c in range(2):
                            nc.sync.dma_start(
                                out=rt[16 * g:16 * (g + 1),
                                       c * RTL:(c + 1) * RTL],
                                in_=tab_ap[32 * r + 16 * c:32 * r + 16 * c + 16, :])
                    for dd0, sp, Ks in buckets:
                        K = Ks[r]
                        ni = sp * K
                        msg = mpool.tile([128, ni], F32, tag="msg")
                        nc.gpsimd.ap_gather(
                            out_ap=msg[:],
                            in_ap=rt[:],
                            idxs_ap=ix_t[:, off:off + ni // 16],
                            channels=128, num_elems=REG, d=1, num_idxs=ni)
                        off += ni // 16
                        tmp = tpool.tile([128, sp], F32, tag="tmp")
                        nc.vector.reduce_sum(
                            out=tmp[:],
                            in_=msg[:].rearrange("p (dd k) -> p dd k", k=K),
                            axis=mybir.AxisListType.X)
                        nc.vector.tensor_add(
                            acc1[:, dd0:dd0 + sp], acc1[:, dd0:dd0 + sp], tmp[:])
                return acc1

            acc1 = aggregate(tab1_ap, ix1_t, 1)
            # post layer 1 (in place): g2 = dv * relu(dv * s1 + b1)
            nc.vector.tensor_mul(acc1[:], acc1[:], dvb_t[:])
            nc.vector.tensor_scalar(
                out=acc1[:], in0=acc1[:], scalar1=b1c_t[:, 0:1], scalar2=None,
                op0=mybir.AluOpType.add)
            nc.scalar.activation(acc1[:], acc1[:],
                                 mybir.ActivationFunctionType.Relu)
            nc.vector.tensor_mul(acc1[:], acc1[:], dvb_t[:])
            for g in range(NGRP):
                nc.sync.dma_start(out=g2t_ap[:, g * BL:(g + 1) * BL],
                                  in_=acc1[16 * g:16 * (g + 1), :])

            nc.gpsimd.collective_compute(
                "AllGather", mybir.AluOpType.bypass, replica_groups=rg,
                ins=[g2t_ap[:, :].opt()], outs=[tab2_ap[:, :].opt()])

            acc2 = aggregate(tab2_ap, ix2_t, 2)
            a2 = acc2
            nc.vector.tensor_mul(a2[:], a2[:], dvb_t[:])

            # ---- phase C: W2 + log_softmax ------------------------------
            a2l = apool.tile([H, BL], F32, tag="a2l")
            for g in range(NGRP):
                # move group slice to partitions 0:16 (PE lhsT/rhs align)
                nc.sync.dma_start(out=a2l[:, :], in_=a2[16 * g:16 * (g + 1), :])
                nsub = (BL + 127) // 128
                for s in range(nsub):
                    c0 = s * 128
                    cw = min(128, BL - c0)
                    lg = psA.tile([C, 128], F32, tag="lg")
                    nc.tensor.matmul(out=lg[:, :cw], lhsT=w2_t[:],
                                     rhs=a2l[:, c0:c0 + cw],
                                     start=True, stop=True)
                    z40 = xtpool.tile([C, 128], F32, tag="z40")
                    nc.any.tensor_copy(z40[:, :cw], lg[:, :cw])
                    ptz = psT.tile([128, 128], F32, tag="ptr")
                    nc.tensor.transpose(out=ptz[:cw, :C], in_=z40[:, :cw],
                                        identity=ident[:C, :C])
                    z = xtpool.tile([128, C], F32, tag="z")
                    nc.vector.tensor_add(z[:cw, :], ptz[:cw, :C], b2_t[:cw, :])
                    mx = xtpool.tile([128, 1], F32, tag="mx")
                    nc.vector.reduce_max(out=mx[:cw, :], in_=z[:cw, :],
                                         axis=mybir.AxisListType.X)
                    nc.vector.tensor_scalar(
                        out=z[:cw, :], in0=z[:cw, :], scalar1=mx[:cw, 0:1],
                        scalar2=None, op0=mybir.AluOpType.subtract)
                    e = xtpool.tile([128, C], F32, tag="e")
                    nc.scalar.activation(e[:cw, :], z[:cw, :],
                                         mybir.ActivationFunctionType.Exp)
                    se = xtpool.tile([128, 1], F32, tag="se")
                    nc.vector.reduce_sum(out=se[:cw, :], in_=e[:cw, :],
                                         axis=mybir.AxisListType.X)
                    ls = xtpool.tile([128, 1], F32, tag="ls")
                    nc.scalar.activation(ls[:cw, :], se[:cw, :],
                                         mybir.ActivationFunctionType.Ln)
                    nc.vector.tensor_scalar(
                        out=z[:cw, :], in0=z[:cw, :], scalar1=ls[:cw, 0:1],
                        scalar2=None, op0=mybir.AluOpType.subtract)
                    nc.sync.dma_start(
                        out=out_ap[g * BL + c0:g * BL + c0 + cw, :],
                        in_=z[:cw, :])

    nc.compile()
    return nc


def _host_prep(x, edge_index, W1, b1, W2, b2, n_cores=M_CORES):
    N, D_IN = x.shape
    H = W1.shape[1]
    C = W2.shape[1]
    NPC = N // n_cores
    NT = (NPC + 127) // 128
    RTL = NT * 128
    BL = RTL // NGRP
    REG = 2 * RTL

    src = np.asarray(edge_index[0], dtype=np.int64)
    dst = np.asarray(edge_index[1], dtype=np.int64)
    deg = np.bincount(dst, minlength=N).astype(np.float64) + 1.0
    dinv = (1.0 / np.sqrt(deg)).astype(np.float32)

    owner = np.minimum(dst // NPC, n_cores - 1)

    # layer-1 table position of node u: owner*RTL + local (natural order)
    own_s = np.minimum(src // NPC, n_cores - 1)

    per_core = []
    for m in range(n_cores):
        sel = owner == m
        s_m = np.concatenate([src[sel], np.arange(m * NPC, (m + 1) * NPC)])
        d_m = np.concatenate([dst[sel] - m * NPC, np.arange(NPC)])
        degl = np.bincount(d_m, minlength=NPC)
        # degree-rank (descending); rank i -> block g=i%8, dd=i//8
        rank_of = np.empty(NPC, dtype=np.int64)
        order = np.argsort(-degl, kind="stable")
        rank_of[order] = np.arange(NPC)
        per_core.append(dict(s_m=s_m, d_m=d_m, degl=degl, rank_of=rank_of,
                             order=order))

    # pos2 (layer-2 table position, block-major): g*BL + dd
    def pos2_of_rank(i):
        return (i % NGRP) * BL + i // NGRP

    pos2 = [pos2_of_rank(pc["rank_of"]) for pc in per_core]  # local->pos2

    # per-dd-position, per-region edge counts; K matrices shared across cores
    # (program is SPMD: single K per bucket across cores too)
    Kmat = np.zeros((NREG, BL), dtype=np.int64)
    counts_all = []
    for m in range(n_cores):
        pc = per_core[m]
        g_arr = pc["rank_of"][pc["d_m"]] % NGRP
        dd_arr = pc["rank_of"][pc["d_m"]] // NGRP
        counts_all.append((g_arr, dd_arr))
    # region of source node u: (owner(u)*RTL + local(u)) // REG  == owner//2
    def region_of(nodes):
        return np.minimum(nodes // NPC, n_cores - 1) // 2

    for m in range(n_cores):
        pc = per_core[m]
        g_arr, dd_arr = counts_all[m]
        r_arr = region_of(pc["s_m"])
        # counts per (region, g, dd)
        lin = (r_arr * NGRP + g_arr) * BL + dd_arr
        cnt = np.bincount(lin, minlength=NREG * NGRP * BL).reshape(
            NREG, NGRP, BL)
        Kmat = np.maximum(Kmat, cnt.max(axis=1))
        pc["cnt"] = cnt
        pc["g_arr"] = g_arr
        pc["dd_arr"] = dd_arr
        pc["r_arr"] = r_arr

    buckets = _make_buckets(None, Kmat)
    XC = sum(sp * k // 16 for dd0, sp, Ks in buckets for k in Ks)

    # per-bucket K lookup per dd position
    Kof = np.zeros((NREG, BL), dtype=np.int64)
    bcol = np.zeros((NREG, BL), dtype=np.int64)   # idx elem offset of (r, dd)
    # column offsets: iterate in emission order (r, then buckets)
    off = 0
    border = []
    for r in range(NREG):
        for dd0, sp, Ks in buckets:
            K = Ks[r]
            border.append((r, dd0, sp, K, off))
            Kof[r, dd0:dd0 + sp] = K
            bcol[r, dd0:dd0 + sp] = off * 16 + (np.arange(dd0, dd0 + sp) - dd0) * K
            off += sp * K // 16
    assert off == XC

    ix1 = np.zeros((n_cores, 128, XC), dtype=np.int16)
    ix2 = np.zeros((n_cores, 128, XC), dtype=np.int16)
    dvb = np.zeros((n_cores, 128, BL), dtype=np.float32)
    b1c = np.zeros((128, 1), dtype=np.float32)
    for g in range(NGRP):
        b1c[16 * g:16 * g + 16, 0] = np.asarray(b1, np.float32)[:H]

    for m in range(n_cores):
        pc = per_core[m]
        # zero-pad source positions (region-local) per layer
        # layer1: column (m%2)*RTL + 12543 is a zeroed pad column of core m
        # (we zero cols NPC..RTL of each core's table)
        # rank of each edge within (dst, region): stable sort by (r, g, dd)
        key = (pc["r_arr"] * NGRP + pc["g_arr"]) * BL + pc["dd_arr"]
        order_e = np.argsort(key, kind="stable")
        srt_src = pc["s_m"][order_e]
        srt_key = key[order_e]
        # rank within same key
        uniq, first_idx, inv = np.unique(srt_key, return_index=True,
                                         return_inverse=True)
        rank_e = np.arange(len(srt_key)) - first_idx[inv]
        r_e = srt_key // (NGRP * BL)
        g_e = (srt_key // BL) % NGRP
        dd_e = srt_key % BL
        K_e = Kof[r_e, dd_e]
        assert (rank_e < K_e).all()
        # idx position jj within instruction (r, bucket): local col
        jj = bcol[r_e, dd_e] + rank_e
        # region-local source positions
        s_own = np.minimum(srt_src // NPC, n_cores - 1)
        s_loc = srt_src - s_own * NPC
        loc1 = (s_own % 2) * RTL + s_loc
        # pos2 per source node (in its own core's numbering)
        p2 = np.empty(len(srt_src), dtype=np.int64)
        for mm in range(n_cores):
            msel = s_own == mm
            p2[msel] = pos2[mm][s_loc[msel]]
        loc2 = (s_own % 2) * RTL + p2
        # fill: start from pad defaults
        pad1 = np.zeros(XC * 16, dtype=np.int16)
        pad2 = np.zeros(XC * 16, dtype=np.int16)
        # pad value: col NPC (first zeroed pad col) of core-half 0
        # layer-2 pad: pos2 of rank NPC (unused position) of half 0
        pad1[:] = NPC          # core half 0, col NPC (zeroed)
        pad2[:] = pos2_of_rank(NPC)
        full1 = np.full((NGRP, XC * 16), NPC, dtype=np.int64)
        full2 = np.full((NGRP, XC * 16), pos2_of_rank(NPC), dtype=np.int64)
        full1[g_e, jj] = loc1
        full2[g_e, jj] = loc2
        # wrap: idx j of group g -> [16g + j%16, j//16]
        w = full1.reshape(NGRP, XC, 16).transpose(0, 2, 1).reshape(128, XC)
        ix1[m] = w.astype(np.int16)
        w = full2.reshape(NGRP, XC, 16).transpose(0, 2, 1).reshape(128, XC)
        ix2[m] = w.astype(np.int16)
        # dv per block position
        dv_loc = np.zeros(RTL, np.float32)
        dv_loc[:NPC] = dinv[m * NPC:(m + 1) * NPC]
        # dvb[16g+h, dd] = dinv[dst(g,dd)] ; dst(g,dd) = order[rank=dd*8+g]
        for g in range(NGRP):
            ranks = np.arange(BL) * NGRP + g
            vals = np.zeros(BL, np.float32)
            ok = ranks < NPC
            vals[ok] = dv_loc[pc["order"][ranks[ok]]]
            dvb[m, 16 * g:16 * g + 16, :] = vals[None, :]

    x_pad = np.zeros((N + RTL, D_IN), np.float32)
    x_pad[:N] = x
    in_maps = []
    for m in range(n_cores):
        in_maps.append({
            "x": np.ascontiguousarray(x_pad[m * NPC:m * NPC + RTL]),
            "w1": np.ascontiguousarray(
                np.asarray(W1, np.float32).reshape(D_IN // 128, 128, H)
                .transpose(1, 0, 2).reshape(128, -1)),
            "w2": np.asarray(W2, np.float32),
            "b1c": b1c,
            "b2": np.tile(np.asarray(b2, np.float32)[None, :], (128, 1)),
            "dvn": _dvn_tile(dinv, m, NPC, NT, RTL),
            "dvb": dvb[m],
            "ix1": ix1[m], "ix2": ix2[m],
        })
    meta = dict(NPC=NPC, NT=NT, RTL=RTL, BL=BL, buckets=buckets,
                orders=[pc["order"] for pc in per_core])
    return in_maps, meta


def _dvn_tile(dinv, m, NPC, NT, RTL):
    nat = np.ones(RTL, np.float32)
    nat[:NPC] = dinv[m * NPC:(m + 1) * NPC]
    return nat.reshape(NT, 128).T.copy()


_CACHE = {}


def _get_program(N, D_IN, H, C, buckets, n_cores=M_CORES):
    NPC = N // n_cores
    NT = (NPC + 127) // 128
    key = (N, D_IN, H, C, str(buckets))
    if key not in _CACHE:
        _CACHE[key] = _build_v3(NT, NPC, D_IN, H, C, buckets, n_cores)
    return _CACHE[key]


def _unpermute(res, meta, N, C, n_cores=M_CORES):
    NPC, BL = meta["NPC"], meta["BL"]
    out = np.empty((N, C), np.float32)
    for m in range(n_cores):
        om = res[m]
        # row pos = g*BL + dd  for rank i = dd*8+g -> local dst order[i]
        i = np.arange(NPC)
        pos = (i % NGRP) * BL + i // NGRP
        out[m * NPC + meta["orders"][m][i]] = om[pos]
    return out


def kernel(x, edge_index, W1, b1, W2, b2):
    x = np.asarray(x)
    n_cores = M_CORES
    N, D_IN = x.shape
    H = np.asarray(W1).shape[1]
    C = np.asarray(W2).shape[1]
    in_maps, meta = _host_prep(x, edge_index, W1, b1, W2, b2, n_cores)
    nc = _get_program(N, D_IN, H, C, meta["buckets"], n_cores)
    res = bass_utils.run_bass_kernel_spmd(nc, in_maps,
                                          core_ids=list(range(n_cores)))
    return _unpermute([res.results[m]["out"] for m in range(n_cores)],
                      meta, N, C, n_cores)
